# revision 6
# baseline (speedup 1.0000x reference)
"""HOCD loss on 8 TRN2 NeuronCores via Bass/Tile.

Full inputs: logits (100, 64, 10000) f32, ref (100, 64) i64, hyp (100, 64) i64.
Data-parallel over batch: core c handles batch columns 8c..8c+7.

Per-core device algorithm (validated against the jax reference in numpy):
  loss[t,b] = LSE(logits[t,b,:]) - (1/|S_tb|) * sum_{c in S_tb} logits[t,b,c]
where S_tb is the set of unique ref tokens r with minimal prefix edit
distance d[t, r] (computed with a tilted-coordinate DP whose deletion-chain
cummin maps to one tensor_tensor_scan per row), LSE uses a zero shift
(logits are O(1), exp is safe in fp32).  Each core returns the partial sum
over its (t, b) of loss/6400; the host adds the 8 partials.
"""
import os
import sys

import numpy as np

if "/opt/trn_rl_repo" not in sys.path:
    sys.path.insert(0, "/opt/trn_rl_repo")

from contextlib import ExitStack

from concourse import bacc, bass, mybir, tile
from concourse.bass_utils import run_bass_kernel_spmd

T, B, R, C = 100, 64, 100, 10000
NCORES = 8
BS = B // NCORES  # 8 batch columns per core
RP = 112          # ref indices padded to a multiple of 16 for ap_gather
INF = 3.0e38
F32 = mybir.dt.float32
I16 = mybir.dt.int16

AF = mybir.ActivationFunctionType
OP = mybir.AluOpType
AX = mybir.AxisListType


def build_nc():
    nc = bacc.Bacc(
        "TRN2",
        target_bir_lowering=False,
        debug=False,
        enable_asserts=False,
        num_devices=NCORES,
    )

    logits_s = nc.dram_tensor("logits_s", [T, BS, C], F32, kind="ExternalInput").ap()
    ref_dp = nc.dram_tensor("ref_dp", [BS, R], F32, kind="ExternalInput").ap()
    hyp_dp = nc.dram_tensor("hyp_dp", [BS, T], F32, kind="ExternalInput").ap()
    refrow = nc.dram_tensor("refrow", [1, BS * R], F32, kind="ExternalInput").ap()
    refcol = nc.dram_tensor("refcol", [R, BS], F32, kind="ExternalInput").ap()
    idx16 = nc.dram_tensor("idx16", [128, BS * (RP // 16)], I16, kind="ExternalInput").ap()
    out_p = nc.dram_tensor("out_p", [1, 1], F32, kind="ExternalOutput").ap()

    with ExitStack() as ctx:
        tc = ctx.enter_context(tile.TileContext(nc, trace_sim=False))
        setup = ctx.enter_context(tc.tile_pool(name="setup", bufs=1))
        bigp = ctx.enter_context(tc.tile_pool(name="bigp", bufs=1))
        dtp = ctx.enter_context(tc.tile_pool(name="dtp", bufs=2))
        dup = ctx.enter_context(tc.tile_pool(name="dup", bufs=2))
        psp = ctx.enter_context(tc.tile_pool(name="psp", bufs=2, space="PSUM"))
        drp = ctx.enter_context(tc.tile_pool(name="drp", bufs=1, space="DRAM"))

        # ---- persistent SBUF state ----
        ref_dp_sb = setup.tile([BS, R], F32, tag="ref_dp_sb")
        hyp_dp_sb = setup.tile([BS, T], F32, tag="hyp_dp_sb")
        refrow_sb = setup.tile([1, BS * R], F32, tag="refrow_sb")
        refcol_sb = setup.tile([R, BS], F32, tag="refcol_sb")
        idx_sb = setup.tile([128, BS * (RP // 16)], I16, tag="idx_sb")
        nc.sync.dma_start(out=ref_dp_sb[:, :], in_=ref_dp)
        nc.sync.dma_start(out=hyp_dp_sb[:, :], in_=hyp_dp)
        nc.sync.dma_start(out=refrow_sb[:, :], in_=refrow)
        nc.sync.dma_start(out=refcol_sb[:, :], in_=refcol)
        nc.sync.dma_start(out=idx_sb[:, :], in_=idx16)

        ones_k1 = setup.tile([1, R], F32, tag="ones_k1")
        nc.gpsimd.memset(ones_k1[:, :], 1.0)
        ones_r = setup.tile([R, 1], F32, tag="ones_r")
        nc.gpsimd.memset(ones_r[:, :], 1.0)

        # iota helpers: jdelrow[p, i] = i ; cmp[p, i] = i - p.
        # f32 iota is imprecise on HW (HW-measured 4e-6 abs err), and these
        # feed exact integer comparisons -> generate int32, convert via copy.
        jdel_i = setup.tile([128, R], mybir.dt.int32, tag="jdel_i")
        nc.gpsimd.iota(jdel_i[:, :], pattern=[[1, R]], base=0, channel_multiplier=0)
        jdelrow = setup.tile([128, R], F32, tag="jdelrow")
        nc.vector.tensor_copy(jdelrow[:, :], jdel_i[:, :])
        cmp_i = setup.tile([128, 128], mybir.dt.int32, tag="cmp_i")
        nc.gpsimd.iota(cmp_i[:, :], pattern=[[1, 128]], base=0, channel_multiplier=-1)
        cmp_t = setup.tile([128, 128], F32, tag="cmp_t")
        nc.vector.tensor_copy(cmp_t[:, :], cmp_i[:, :])
        tri = setup.tile([128, 128], F32, tag="tri")
        nc.vector.tensor_single_scalar(tri[:, :], cmp_t[:, :], 0.0, OP.is_gt)
        ident = setup.tile([128, 128], F32, tag="ident")
        nc.vector.tensor_single_scalar(ident[:, :], cmp_t[:, :], 0.0, OP.is_equal)

        # big double-buffered logits blocks; pad rows [T:128] once so
        # ap_gather never reads uninitialized SBUF
        big = [
            bigp.tile([128, C], F32, tag=f"big{i}", name=f"big{i}") for i in range(2)
        ]
        for i in range(2):
            nc.gpsimd.memset(big[i][96:128, :], 0.0)
        expscr = bigp.tile([T, C], F32, tag="expscr")
        G_all = setup.tile([128, BS * RP], F32, tag="G_all")
        escol = setup.tile([T, BS], F32, tag="escol")
        gscol = setup.tile([T, BS], F32, tag="gscol")
        ccol = setup.tile([T, BS], F32, tag="ccol")

        # ---- phase A: stream logits; exp+rowsum on ACT; token gather on POOL
        for b in range(BS):
            bt = big[b % 2]
            nc.sync.dma_start(out=bt[0:T, :], in_=logits_s[:, b, :])
            nc.scalar.activation(expscr[:, :], bt[0:T, :], AF.Exp,
                                 accum_out=escol[:, b : b + 1])
            nc.gpsimd.ap_gather(
                out_ap=G_all[:, b * RP : (b + 1) * RP],
                in_ap=bt[:, :],
                idxs_ap=idx_sb[:, b * (RP // 16) : (b + 1) * (RP // 16)],
                channels=128,
                num_elems=C,
                d=1,
                num_idxs=RP,
            )

        # ---- DP (DVE), tilted coords: U[t,j] = d[t,j] - j ----
        Urows = setup.tile([BS, T, R + 1], F32, tag="Urows")
        Vbuf = setup.tile([BS, R + 1], F32, tag="Vbuf")
        P1buf = setup.tile([BS, R + 1], F32, tag="P1buf")
        eqbuf = setup.tile([BS, R], F32, tag="eqbuf")
        nc.vector.memset(Urows[:, 0, :], 0.0)
        nc.vector.memset(Vbuf[:, 0:1], INF)
        for t in range(1, T):
            h = hyp_dp_sb[:, t - 1 : t]
            Uprev = Urows[:, t - 1, :]
            nc.vector.tensor_single_scalar(eqbuf[:, :], ref_dp_sb[:, :], h, OP.is_equal)
            nc.vector.tensor_tensor(Vbuf[:, 1 : R + 1], Uprev[:, 0:R], eqbuf[:, :], OP.subtract)
            nc.vector.tensor_single_scalar(P1buf[:, :], Uprev, 1.0, OP.add)
            nc.vector.tensor_tensor_scan(
                Urows[:, t, :], P1buf[:, :], Vbuf[:, :],
                initial=INF, op0=OP.min, op1=OP.min,
            )

        # bounce DP rows through DRAM to flip (b-part, t-free) -> (t-part)
        dpd = drp.tile([BS, T, R + 1], F32, tag="dpd")
        nc.scalar.dma_start(out=dpd[:, :, :], in_=Urows[:, :, :])

        # ---- phase B: per-b optimal-set extraction + dedup + weighted gather
        ubuf = setup.tile([T, RP], F32, tag="ubuf")
        nc.vector.memset(ubuf[:, R:RP], 0.0)
        scrap = setup.tile([T, RP], F32, tag="scrap")
        for b in range(BS):
            Dt = dtp.tile([T, R + 1], F32, tag="dt")
            nc.scalar.dma_start(out=Dt[:, :], in_=dpd[b, :, :])
            DU = dup.tile([T, R], F32, tag="du")
            nc.vector.tensor_tensor(DU[:, :], Dt[:, 0:R], jdelrow[0:T, :], OP.add)
            mn = dup.tile([T, 1], F32, tag="mn")
            nc.vector.tensor_reduce(mn[:, :], DU[:, :], AX.X, OP.min)
            u0 = dup.tile([T, R], F32, tag="u0")
            nc.vector.tensor_single_scalar(u0[:, :], DU[:, :], mn[:, :], OP.is_equal)

            rr_ps = psp.tile([R, R], F32, tag="rr_ps")
            nc.tensor.matmul(rr_ps[:, :], ones_k1[:, :],
                             refrow_sb[:, b * R : (b + 1) * R], start=True, stop=True)
            E_sb = dup.tile([R, R], F32, tag="e_sb")
            nc.vector.scalar_tensor_tensor(
                E_sb[:, :], rr_ps[:, :], refcol_sb[:, b : b + 1], tri[0:R, 0:R],
                op0=OP.is_equal, op1=OP.mult,
            )
            u0T_ps = psp.tile([R, T], F32, tag="u0t_ps")
            nc.tensor.transpose(u0T_ps[:, :], u0[:, :], ident[0:T, 0:R])
            u0T_sb = dup.tile([R, T], F32, tag="u0t_sb")
            nc.vector.tensor_copy(u0T_sb[:, :], u0T_ps[:, :])
            bad_ps = psp.tile([T, R], F32, tag="bad_ps")
            nc.tensor.matmul(bad_ps[:, :], u0T_sb[:, :], E_sb[:, :],
                             start=True, stop=True)
            nc.vector.scalar_tensor_tensor(
                ubuf[:, 0:R], bad_ps[:, :], 0.5, u0[:, :],
                op0=OP.is_lt, op1=OP.mult,
            )
            nc.vector.tensor_reduce(ccol[:, b : b + 1], ubuf[:, :], AX.X, OP.add)
            nc.vector.tensor_tensor(
                scrap[:, :], G_all[0:T, b * RP : (b + 1) * RP], ubuf[:, :], OP.mult
            )
            nc.vector.tensor_reduce(gscol[:, b : b + 1], scrap[:, :], AX.X, OP.add)

        # ---- finale ----
        lse = setup.tile([T, BS], F32, tag="lse")
        nc.scalar.activation(lse[:, :], escol[:, :], AF.Ln)
        rc = setup.tile([T, BS], F32, tag="rc")
        nc.vector.reciprocal(rc[:, :], ccol[:, :])
        tmp = setup.tile([T, BS], F32, tag="tmp")
        nc.vector.tensor_tensor(tmp[:, :], gscol[:, :], rc[:, :], OP.mult)
        lossv = setup.tile([T, BS], F32, tag="lossv")
        nc.vector.tensor_tensor(lossv[:, :], lse[:, :], tmp[:, :], OP.subtract)
        s1 = setup.tile([T, 1], F32, tag="s1")
        nc.vector.tensor_reduce(s1[:, :], lossv[:, :], AX.X, OP.add)
        tot_ps = psp.tile([1, 1], F32, tag="tot_ps")
        nc.tensor.matmul(tot_ps[:, :], ones_r[:, :], s1[:, :], start=True, stop=True)
        outsb = setup.tile([1, 1], F32, tag="outsb")
        nc.scalar.activation(outsb[:, :], tot_ps[:, :], AF.Copy, scale=1.0 / (T * B))
        nc.sync.dma_start(out=out_p, in_=outsb[:, :])

    nc.compile()
    return nc


def make_in_maps(logits, ref, hyp):
    logits = np.asarray(logits, np.float32)
    ref = np.asarray(ref).astype(np.int64)
    hyp = np.asarray(hyp).astype(np.int64)
    in_maps = []
    for c in range(NCORES):
        bsl = slice(c * BS, (c + 1) * BS)
        ref_c = ref[:, bsl]  # (R, BS)
        hyp_c = hyp[:, bsl]  # (T, BS)
        idx = np.zeros((128, BS * (RP // 16)), np.int16)
        for b in range(BS):
            L = np.zeros(RP, np.int16)
            L[:R] = ref_c[:, b].astype(np.int16)
            w = np.zeros((16, RP // 16), np.int16)
            for r in range(RP):
                w[r % 16, r // 16] = L[r]
            for g in range(8):
                idx[16 * g : 16 * (g + 1), b * (RP // 16) : (b + 1) * (RP // 16)] = w
        in_maps.append(
            {
                "logits_s": np.ascontiguousarray(logits[:, bsl, :]),
                "ref_dp": np.ascontiguousarray(ref_c.T.astype(np.float32)),
                "hyp_dp": np.ascontiguousarray(hyp_c.T.astype(np.float32)),
                "refrow": np.ascontiguousarray(
                    ref_c.T.astype(np.float32).reshape(1, BS * R)
                ),
                "refcol": np.ascontiguousarray(ref_c.astype(np.float32)),
                "idx16": idx,
            }
        )
    return in_maps


_NC_CACHE = {}


def get_nc():
    if "nc" not in _NC_CACHE:
        _NC_CACHE["nc"] = build_nc()
    return _NC_CACHE["nc"]


def kernel(logits, ref, hyp):
    nc = get_nc()
    in_maps = make_in_maps(logits, ref, hyp)
    res = run_bass_kernel_spmd(nc, in_maps, core_ids=list(range(NCORES)))
    total = np.float32(0.0)
    for c in range(NCORES):
        total += np.float32(res.results[c]["out_p"][0, 0])
    return np.array(total, dtype=np.float32)


if __name__ == "__main__":
    import reference as refmod

    inputs = refmod.setup_inputs()
    expected = np.asarray(refmod.reference(**inputs))
    actual = kernel(
        np.asarray(inputs["logits"]), np.asarray(inputs["ref"]), np.asarray(inputs["hyp"])
    )
    rel = abs(float(actual) - float(expected)) / max(abs(float(expected)), 1e-12)
    print(f"expected={expected} actual={actual} rel={rel:.3e}")
